# revision 37
# baseline (speedup 1.0000x reference)
"""Multi-head attention TRN2 kernel, head-parallel over 8 NeuronCores.

Reference computation (fp32):
    q,k,v = x@Wq, x@Wk, x@Wv          # [B,S,16*64]
    attn  = softmax(q k^T / 8)         # per head
    out   = (attn @ v) @ Wo            # [B,S,1024]

Sharding: tensor-parallel over heads. Core c owns heads (2c, 2c+1):
Wq/Wk/Wv columns [128c:128c+128], Wo rows [128c:128c+128]. Each core
produces a full-shape partial output; the host sums the 8 partials.

v2 design notes (vs the bf16 v1 baseline):
- Projections run as fp8e4m3 DoubleRow matmuls with hi+lo error
  compensation (x = x_hi + x_lo, W = W_hi + W_lo; the lo*lo cross term
  is dropped), giving ~bf16 precision at ~40% of the bf16 PE cost.
- Scores use a q-compensated DoubleRow matmul: lhsT is k8 read through a
  stride-0 pair view, rhs is the (q_hi, q_lo) pair, so s = k8^T(q_hi+q_lo).
  Only the k-side fp8 quantization error survives (~1e-2 of absmax) at
  half the bf16 PE cost.
- exp(s/8) runs on BOTH the ACT engine (native Exp) and the DVE (custom
  8-stage fused op evaluating the minimax polynomial ((a*s+b)^2+c)^16,
  <0.9% rel err over |s/8|<=3.8), splitting the softmax elementwise wall.
  Score PSUM tiles are paired [128,1024] so each exp instruction covers
  two key-chunks (amortizes fixed overheads).
- AV runs "flipped" ([queries, v-dims] output) in bf16: full 128-wide
  partition utilization halves its PE cost vs the v1 orientation, and the
  softmax denominator (ones-column of V) lands per-PARTITION, so the
  normalize is one reciprocal + one tensor_scalar per head-chunk.
- Normalized attention output transposes back to [inner, seq] via PE
  transpose (identity staged from the host) for the bf16 out-proj.
- GPSIMD cannot touch PSUM on TRN2, so all PSUM drains are split across
  ACT (activation-Copy) and DVE; projection work for batch b+1 is
  interleaved into batch b's attention units to keep the PE busy while
  ACT/DVE chew exps.

v3 notes (283us -> 261us on the calibrated timeline):
- The kernel is PSUM-egress-bound: every score/projection/AV element must
  leave PSUM through ACT (1.2G elem/s) or DVE (0.96G), ~430us of combined
  drain+exp work split ~215us/engine. All elementwise ops were widened to
  amortize the fixed access-latency charge: one strided quad-normalize +
  one quad-reciprocal per unit (was 4+4), one 512-wide transpose drain per
  ic (was 4x128 — bf16 PSUM reads get the DVE 2x mode), paired V drains.
- DMA dispatch on SP.SEQ costs ~0.8us per descriptor chain, so the 6 fp8
  weight slabs ship as one dram tensor/DMA (wq+wk first), and x ships as
  one DMA per tensor per batch. Batch 0 loads x in per-ic column windows
  so each K/Q projection chain unblocks as its window lands; first exp
  starts ~10us (was ~20us).
- Score psum tiles (2 bufs x 2 banks) create a score->exp->score cycle of
  ~730ns/pair; the weave + priority tuning (exps high-priority, drains
  natural order, out-proj matmuls slightly late, DVE pairs {1,4,6}) keeps
  both elementwise engines ~93% packed mid-kernel. The final unit's exp
  split is rebalanced (4 DVE/4 ACT) so both engines finish together, and
  the out-proj chunks of an ic spread across 2 gus.
"""

from contextlib import ExitStack
import dataclasses

import numpy as np

HEADS = 16
DH = 64
D = 1024
B = 4
S = 2048
N_CORES = 8
HPC = HEADS // N_CORES  # heads per core = 2

# minimax fit of ((a*y + b)^2 + c)^16 ~ exp(y) over y in [-3.8, 3.8],
# max rel err 0.89%. y = score/8 is folded into C0 (raw scores in).
_PA = 0.044116274207348274
_PB = 0.7133243299023739
_PC = 0.4912647012807798

# fp8 pre-scales: e4m3 subnormals start at 2^-6, so small-sigma data (W has
# sigma=0.02) must be scaled into the normal range before quantization. The
# scales fold into the PSUM drain copies and the exp() scale.
XS = 8.0    # x pre-scale
WS = 32.0   # W pre-scale
QS = 4.0    # q/k drain re-scale
QK_DRAIN = QS / (XS * WS)          # psq -> q/k fp8
V_DRAIN = 1.0 / (XS * WS)          # psv -> V bf16
SSC = 1.0 / (QS * QS)              # score psum = (1/SSC) * raw score
EXP_SCALE = 0.125 * SSC            # ACT exp scale on score psum
C0R = _PA * EXP_SCALE / 2**0.5 * 2**0.5  # == _PA/8*SSC, kept explicit below
C0R = _PA * 0.125 * SSC

# score key-chunk-PAIR indices whose exp runs on the DVE custom op
DVE_JCPS = frozenset({1, 4, 6})  # of 8 pairs per unit
DVE_JCPS_LAST = frozenset({0, 3, 6})  # split for the final batch (no proj drains)
DVE_JCPS_LAST_ALT = None  # if set, odd units of the final batch use this set
OP_LAG_LAST = None  # op lag override for the final ic (None = OP_LAG)
DVE_JCPS_H1 = None  # optional override for h==1 units (they add tr work to DVE)
DVE_JCPS_FIRST = frozenset({0, 3, 5, 6})  # batch-0 early units: DVE has slack
FIRST_UNITS = 2   # how many leading batch-0 units use DVE_JCPS_FIRST
SPLIT_JCPS = frozenset()  # pairs exp'd half-on-ACT half-on-DVE (faster drain)
EXTRA_DVE_EVERY = 4       # every Nth unit sends EXTRA_DVE_PAIRS to DVE too
EXTRA_DVE_PAIRS = frozenset({4})
# engine assignment knobs for the PSUM drains ("act" or "dve")
ENG_V = "dve"
ENG_NORM = "dve"
ENG_TR = "dve"
ENG_OP1 = "dve"
ENG_QLO = "dve"
SCORE_PAIR = True   # pair two key-chunks per PSUM tile / exp instr
PSS_BUFS = 2
PSM_BUFS = 2
HIPRI_EXP = True
SPLIT_PSS = False   # separate score-psum pools for ACT vs DVE exp chains
TR_MODE = "pe"      # "dma" (XBAR) or "pe" (PE transpose + DVE copy)
OP_LAG = 1          # gus between transpose and its out-proj
PSO_QUAD = False    # pack 4 qc AV accumulators into one PSUM bank tile
EXB_BUFS = 5
AN_BUFS = 2
OB_BUFS = 5
PROJ_MODE = "full"   # "full" = 3-term hi/lo comp; "xcomp" = (x_hi+x_lo)*W_hi
VPROJ_MODE = "full"  # same for the V projection
OP_PAIR = False      # pair both oc halves into one [128,1024] psum + 1 drain
WEAVE_MODE = 1       # 0: A then O at odd slots; 1: O first; 2: proj woven too
FILL_SPLIT = False   # defer batch-0 Q ic1..3 into the first gus
DRAIN_PRI = "off"   # "zero": priority 0; "off": natural; int: offset
SCORE_PRI = 100000   # scores hoisted first, relative order preserved
AV_PRI = None        # AV matmul priority ("zero" | int offset | None)
PROJ_PRI = None      # projection matmul priority (negative offset = later)
TR_PRI = None        # PE transpose priority
OPMM_PRI = -100      # out-proj matmul priority
XLOAD_PRI = None     # x-load DMA priority
OP_SPREAD = True    # spread the 4 op chunks of an ic across 2 gus
QK_SPREAD = False    # K projections in gus 0..3, Q in gus 4..7 (1 per gu)
TAIL_PSS = False     # final-ic op psum from the (idle) score pool
PROJ_CARRY = False   # self-carry Q1-3/V12-15 in own window; smaller preamble
OP_OC_ACT = 0        # which out-proj oc half drains on ACT (other goes DVE)
DVE_JCPS_FINAL_UNIT = frozenset({0, 2, 4, 6})  # exp split override for the very last unit

_EXP_OP = None


def _exp_op():
    """Register (once) the fused DVE op computing the exp polynomial."""
    global _EXP_OP
    if _EXP_OP is not None:
        return _EXP_OP
    from concourse import dve_ops
    from concourse.dve_spec import C0, C1, C2, Spec, Src0, lower, sq
    from concourse.dve_uop import DveOpSpec

    name = "EXP_POLY16_ANT"
    if name in dve_ops._SUB_OPCODE_FOR_NAME:
        _EXP_OP = next(op for op in dve_ops.OPS if op.name == name)
        return _EXP_OP
    w = Src0 * C0 + C1
    base = sq(w) + C2
    body = sq(sq(sq(sq(base))))
    spec = Spec(
        body=body,
        reference=lambda in0, in1, c0, c1, c2: (
            ((in0.astype(np.float32) * c0 + c1) ** 2 + c2) ** 16
        ),
    )
    opcode = max(dve_ops._SUB_OPCODE_FOR_NAME.values()) + 1
    assert opcode < 0x20
    shas = {
        ver: DveOpSpec(
            name=name, opcode=opcode, uops=lower(spec, ver=ver), rd1_en=False
        ).sha(ver)
        for ver in ("v3", "v4")
    }
    dve_ops._SUB_OPCODE_FOR_NAME[name] = opcode
    op = dve_ops.DveOp(name, spec, subdim=False, uops_sha=shas)
    dve_ops.OPS.append(op)
    dve_ops.CUSTOM_DVE_SPECS[name] = spec
    _EXP_OP = op
    return op


def _pairdim(ap, n=2):
    """Insert a stride-0 dim of size n after the partition dim (broadcast)."""
    return dataclasses.replace(ap, ap=[ap.ap[0], [0, n], *ap.ap[1:]])


def _bcast_inner(ap, n):
    """Append a stride-0 inner dim of size n (broadcast along free)."""
    return dataclasses.replace(ap, ap=[*ap.ap, [0, n]])


def _strided2(ap, stride, count, inner):
    """Reshape a [P, F] AP into [P, count, inner] with the given outer stride."""
    return dataclasses.replace(ap, ap=[ap.ap[0], [stride, count], [1, inner]])


def build_attention_kernel(nc, b=B, s=S):
    """Emit the per-core program. b/s shrinkable for simulator testing."""
    import concourse.bass as bass
    import concourse.tile as tile
    from concourse import mybir

    bf16 = mybir.dt.bfloat16
    f8 = mybir.dt.float8e4
    f32 = mybir.dt.float32
    ts = bass.ts
    DR = mybir.MatmulPerfMode.DoubleRow
    Exp = mybir.ActivationFunctionType.Exp
    Copy = mybir.ActivationFunctionType.Copy
    mult = mybir.AluOpType.mult
    subtract = mybir.AluOpType.subtract
    EXP_OP = _exp_op()

    DC = D // 128          # D chunks of 128 (contraction tiles)
    IC = s // 512          # query chunks of 512 per batch
    JC = s // 128          # key chunks of 128 per batch
    JP = JC // 2           # key-chunk pairs
    SC = s // 128          # seq chunks of 128
    OC = D // 512          # output-dim chunks of 512
    NU = 2 * IC            # attention units per batch: (ic, h)

    xhi_d = nc.dram_tensor("xhi", [D, b * s], f8, kind="ExternalInput").ap()
    xlo_d = nc.dram_tensor("xlo", [D, b * s], f8, kind="ExternalInput").ap()
    # all six fp8 weight slabs in one tensor -> one DMA dispatch
    w6_d = nc.dram_tensor("w6", [128, 6, DC, 128], f8, kind="ExternalInput").ap()
    wo_d = nc.dram_tensor("wo", [128, D], bf16, kind="ExternalInput").ap()
    id_d = nc.dram_tensor("ident", [128, 128], bf16, kind="ExternalInput").ap()
    out_d = nc.dram_tensor("out_p", [b * s, D], bf16, kind="ExternalOutput").ap()

    with tile.TileContext(nc) as tc, ExitStack() as ctx:
        from contextlib import nullcontext

        def _dp():
            if DRAIN_PRI == "off":
                return nullcontext()
            if DRAIN_PRI == "zero":
                return tc.high_priority()
            return tc.high_priority(offset=DRAIN_PRI)

        def _pri(v):
            if v is None:
                return nullcontext()
            if v == "zero":
                return tc.high_priority()
            return tc.high_priority(offset=v)

        wpool = ctx.enter_context(tc.tile_pool(name="weights", bufs=1))
        xpool = ctx.enter_context(tc.tile_pool(name="x", bufs=2))
        qkpool = ctx.enter_context(tc.tile_pool(name="qk", bufs=2))
        vpool = ctx.enter_context(tc.tile_pool(name="v", bufs=2))
        otpool = ctx.enter_context(tc.tile_pool(name="ot", bufs=2))
        expool = ctx.enter_context(tc.tile_pool(name="exp", bufs=EXB_BUFS))
        anpool = ctx.enter_context(tc.tile_pool(name="an", bufs=AN_BUFS))
        smpool = ctx.enter_context(tc.tile_pool(name="small", bufs=8))
        obpool = ctx.enter_context(tc.tile_pool(name="ob", bufs=OB_BUFS))
        # PSUM: 8 banks total. psm (1 bank x2) is shared by projections,
        # out-proj and the attention transpose; pss holds jc-PAIR score
        # tiles (2 banks x2); pso holds the flipped-AV accumulators.
        ps_mm = ctx.enter_context(tc.tile_pool(name="psm", bufs=PSM_BUFS, space="PSUM"))
        if SPLIT_PSS:
            ps_sA = ctx.enter_context(tc.tile_pool(name="pssA", bufs=1, space="PSUM"))
            ps_sD = ctx.enter_context(tc.tile_pool(name="pssD", bufs=1, space="PSUM"))
        else:
            ps_s = ctx.enter_context(tc.tile_pool(name="pss", bufs=PSS_BUFS, space="PSUM"))
            ps_sA = ps_sD = ps_s
        ps_o = ctx.enter_context(tc.tile_pool(name="pso", bufs=2, space="PSUM"))
        if OP_PAIR:
            ps_p2 = ctx.enter_context(tc.tile_pool(name="psp2", bufs=1, space="PSUM"))

        w6_sb = wpool.tile([128, 6, DC, 128], f8, tag="w6", name="w6")
        # wq+wk first so the batch-0 K/Q chains start before wv lands
        nc.sync.dma_start(w6_sb[:, 0:4], w6_d[:, 0:4])
        nc.sync.dma_start(w6_sb[:, 4:6], w6_d[:, 4:6])
        w_sb = {}
        for i, key in enumerate(
            [(wn, p) for wn in ("wq", "wk", "wv") for p in ("hi", "lo")]
        ):
            w_sb[key] = w6_sb[:, i]
        wo_sb = wpool.tile([128, D], bf16, tag="wo")
        nc.sync.dma_start(wo_sb[:], wo_d[:])
        ident = wpool.tile([128, 128], bf16, tag="ident")
        nc.sync.dma_start(ident[:], id_d[:])

        # ---------------- per-batch state handles ----------------
        xb = {}      # bi -> (xhi tile, xlo tile)
        qk = {}      # bi -> (QT8 [128,2,s], K8 [128,s])
        vt = {}      # bi -> V
        vps = {}     # bi -> pending V-projection psum pair tile
        ott = {}     # bi -> OT
        exb = {}     # (bi, u) -> exB
        attn = {}    # (bi, ic) -> attn_nat quad tile
        psos = {}    # (bi, u) -> quad AV psum tile

        def _xdram(xd, dc0, ndc, bi):
            """DRAM view [128, ndc, s] of x chunks dc0..dc0+ndc for batch bi."""
            base = xd[dc0 * 128 : dc0 * 128 + 128, bi * s : (bi + 1) * s]
            return dataclasses.replace(
                base, ap=[base.ap[0], [128 * b * s, ndc], base.ap[1]]
            )

        def load_x(bi, fine=False):
            """fine=True: per-ic column-window DMAs so the batch-0 K/Q
            projection chains unblock progressively; else one DMA per tensor
            (single dispatch, latency hidden behind compute)."""
            xh = xpool.tile([128, DC, s], f8, tag="xh")
            xl = xpool.tile([128, DC, s], f8, tag="xl")
            with _pri(XLOAD_PRI):
                if fine:
                    for ic in range(IC):
                        cw = slice(ic * 512, (ic + 1) * 512)
                        for t, d in ((xh, xhi_d), (xl, xlo_d)):
                            src = _xdram(d, 0, DC, bi)
                            src = dataclasses.replace(
                                src,
                                offset=src.offset + ic * 512,
                                ap=[src.ap[0], src.ap[1], [1, 512]],
                            )
                            nc.sync.dma_start(t[:, :, cw], src)
                else:
                    nc.sync.dma_start(xh[:], _xdram(xhi_d, 0, DC, bi))
                    nc.sync.dma_start(xl[:], _xdram(xlo_d, 0, DC, bi))
            xb[bi] = (xh, xl)

        def comp_steps(whi, wlo, xh, xl, cols, full=True):
            """DR step list for the compensated projection contraction.
            full: (W_hi+W_lo)x_hi + W_hi x_lo; else W_hi(x_hi + x_lo) only
            (one fp8 W quantization leg survives, ~7e-3 of absmax)."""
            steps = []
            for dcp in range(DC // 2):
                dsl = slice(2 * dcp, 2 * dcp + 2)
                steps.append((whi[:, dsl, :], xh[:, dsl, cols]))
                if full:
                    steps.append((wlo[:, dsl, :], xh[:, dsl, cols]))
                steps.append((whi[:, dsl, :], xl[:, dsl, cols]))
            return steps

        def proj_qk(bi, ic, which=("wq", "wk")):
            """Q and/or K projection for query-chunk ic of batch bi."""
            xh, xl = xb[bi]
            if bi not in qk:
                QT8 = qkpool.tile([128, 2, s], f8, tag="qt")
                K8 = qkpool.tile([128, s], f8, tag="kt")
                qk[bi] = (QT8, K8)
            QT8, K8 = qk[bi]
            for wn in which:
                psq = ps_mm.tile([128, 512], f32, tag="psm")
                steps = comp_steps(
                    w_sb[wn, "hi"], w_sb[wn, "lo"], xh, xl, ts(ic, 512),
                    full=(PROJ_MODE == "full"),
                )
                n = len(steps)
                with _pri(PROJ_PRI):
                    for i, (lhsT, rhs) in enumerate(steps):
                        nc.tensor.matmul(
                            psq[:], lhsT=lhsT, rhs=rhs,
                            start=(i == 0), stop=(i == n - 1), perf_mode=DR,
                        )
                with _dp():
                    if wn == "wq":
                        nc.scalar.activation(
                            QT8[:, 0, ts(ic, 512)], psq[:], Copy, scale=QK_DRAIN
                        )
                        nc.vector.scalar_tensor_tensor(
                            QT8[:, 1, ts(ic, 512)], psq[:], QK_DRAIN,
                            QT8[:, 0, ts(ic, 512)], mult, subtract,
                        )
                    else:
                        nc.scalar.activation(
                            K8[:, ts(ic, 512)], psq[:], Copy, scale=QK_DRAIN
                        )

        def proj_v(bi, sc):
            """V projection for key-chunk sc of batch bi (natural layout).
            sc pairs share one psum tile; the drain fires on the odd sc as a
            single 256-row strided copy."""
            xh, xl = xb[bi]
            if sc == 0:
                V = vpool.tile([128, SC, 130], bf16, tag="v")
                nc.vector.memset(V[:, :, 64:65], 1.0)
                nc.vector.memset(V[:, :, 129:130], 1.0)
                vt[bi] = V
            V = vt[bi]
            if sc % 2 == 0:
                vps[bi] = ps_mm.tile([128, 256], f32, tag="psm", name="psv")
            psv = vps[bi]
            half = sc % 2
            steps = []
            for dcp in range(DC // 2):
                dsl = slice(2 * dcp, 2 * dcp + 2)
                steps.append((xh[:, dsl, ts(sc, 128)], w_sb["wv", "hi"][:, dsl, :]))
                if VPROJ_MODE == "full":
                    steps.append((xh[:, dsl, ts(sc, 128)], w_sb["wv", "lo"][:, dsl, :]))
                steps.append((xl[:, dsl, ts(sc, 128)], w_sb["wv", "hi"][:, dsl, :]))
            n = len(steps)
            with _pri(PROJ_PRI):
                for i, (lhsT, rhs) in enumerate(steps):
                    nc.tensor.matmul(
                        psv[:, half * 128 : half * 128 + 128], lhsT=lhsT, rhs=rhs,
                        start=(i == 0), stop=(i == n - 1), perf_mode=DR,
                    )
            if half == 0:
                return
            # one strided scaled copy for both sc chunks:
            # psum [128, (2 sc, 2, 64)] -> V[:, sc-1:sc+1, (0:64, 65:129)]
            vps.pop(bi)
            vv = V[:, sc - 1 : sc + 1, 0:129]
            vv = dataclasses.replace(
                vv, ap=[vv.ap[0], vv.ap[1], [65, 2], [1, 64]]
            )
            pv = dataclasses.replace(
                psv[:], ap=[psv[:].ap[0], [128, 2], [64, 2], [1, 64]]
            )
            with _dp():
                if ENG_V == "dve":
                    nc.vector.tensor_scalar(vv, pv, V_DRAIN, None, mult)
                else:
                    nc.scalar.activation(vv, pv, Copy, scale=V_DRAIN)

        def scores_exp_chunk(bi, u, jp):
            """One score-pair (2 key-chunks) + its exp, for unit u=(ic,h)."""
            ic, h = divmod(u, 2)
            QT8, K8 = qk[bi]
            hs = h * 64
            if jp == 0:
                exB = expool.tile([128, JP, 1024], bf16, tag="ex", name=f"ex{bi}_{u}")
                exb[bi, u] = exB
            exB = exb[bi, u]
            from contextlib import nullcontext

            def pri():
                if HIPRI_EXP is True:
                    return tc.high_priority()
                if HIPRI_EXP:
                    return tc.high_priority(offset=HIPRI_EXP)
                return nullcontext()
            if SCORE_PAIR:
                dve = jp in DVE_JCPS
                pool = ps_sD if dve else ps_sA
                pss = pool.tile([128, 1024], f32, tag="pssD" if (dve and SPLIT_PSS) else "pss")
                if SCORE_PRI is None:
                    spri = nullcontext()
                elif SCORE_PRI == "zero":
                    spri = tc.high_priority()
                else:
                    spri = tc.high_priority(offset=SCORE_PRI)
                with spri:
                    for half in range(2):
                        jc = 2 * jp + half
                        nc.tensor.matmul(
                            pss[:, half * 512:(half + 1) * 512],
                            lhsT=_pairdim(K8[hs:hs + 64, ts(jc, 128)]),
                            rhs=QT8[hs:hs + 64, :, ts(ic, 512)],
                            start=True, stop=True, perf_mode=DR,
                        )
                jset = DVE_JCPS_LAST if bi == b - 1 else DVE_JCPS
                if bi == b - 1 and u % 2 == 1 and DVE_JCPS_LAST_ALT is not None:
                    jset = DVE_JCPS_LAST_ALT
                if (bi == b - 1 and u == NU - 1
                        and DVE_JCPS_FINAL_UNIT is not None):
                    jset = DVE_JCPS_FINAL_UNIT
                if h == 1 and DVE_JCPS_H1 is not None:
                    jset = DVE_JCPS_H1
                if bi == 0 and u < FIRST_UNITS and DVE_JCPS_FIRST is not None:
                    jset = DVE_JCPS_FIRST
                if EXTRA_DVE_EVERY and (bi * NU + u) % EXTRA_DVE_EVERY == 0:
                    jset = frozenset(jset) | EXTRA_DVE_PAIRS
                with pri():
                    if jp in SPLIT_JCPS:
                        nc.scalar.activation(
                            exB[:, jp, 0:512], pss[:, 0:512], Exp, scale=EXP_SCALE
                        )
                        nc.vector._custom_dve(
                            EXP_OP, out=exB[:, jp, 512:1024], in0=pss[:, 512:1024],
                            s0=C0R, s1=_PB, imm2=_PC,
                        )
                    elif jp in jset:
                        nc.vector._custom_dve(
                            EXP_OP, out=exB[:, jp, :], in0=pss[:],
                            s0=C0R, s1=_PB, imm2=_PC,
                        )
                    else:
                        nc.scalar.activation(
                            exB[:, jp, :], pss[:], Exp, scale=EXP_SCALE
                        )
            else:
                for half in range(2):
                    jc = 2 * jp + half
                    pss = ps_s.tile([128, 512], f32, tag="pss")
                    nc.tensor.matmul(
                        pss[:],
                        lhsT=_pairdim(K8[hs:hs + 64, ts(jc, 128)]),
                        rhs=QT8[hs:hs + 64, :, ts(ic, 512)],
                        start=True, stop=True, perf_mode=DR,
                    )
                    jset = DVE_JCPS_LAST if bi == b - 1 else DVE_JCPS
                    with pri():
                        if jp in jset:
                            nc.vector._custom_dve(
                                EXP_OP,
                                out=exB[:, jp, half * 512:(half + 1) * 512],
                                in0=pss[:], s0=C0R, s1=_PB, imm2=_PC,
                            )
                        else:
                            nc.scalar.activation(
                                exB[:, jp, half * 512:(half + 1) * 512],
                                pss[:], Exp, scale=EXP_SCALE,
                            )

        def av_norm_chunk(bi, u, qc):
            """Flipped AV for one query sub-chunk of unit u. At qc==3 the
            whole unit normalizes in one strided quad op; at (h==1, qc==3)
            the finished [q, inner] ic-block transposes into OT via 4 PE
            transposes + one 512-wide drain."""
            ic, h = divmod(u, 2)
            V = vt[bi]
            exB = exb[bi, u]
            hs = h * 64
            if ic == 0 and h == 0 and qc == 0:
                OT = otpool.tile([128, s], bf16, tag="ot")
                ott[bi] = OT
            if qc == 0:
                psoq = ps_o.tile([128, 4, 65], f32, tag="pso", name="psoq")
                psos[bi, u] = psoq
            psoq = psos[bi, u]
            with _pri(AV_PRI):
                for jc in range(JC):
                    jp, half = divmod(jc, 2)
                    nc.tensor.matmul(
                        psoq[:, qc, :],
                        lhsT=exB[:, jp, half * 512 + qc * 128: half * 512 + (qc + 1) * 128],
                        rhs=V[:, jc, 65 * h : 65 * h + 65],
                        start=(jc == 0), stop=(jc == JC - 1),
                    )
            if qc < 3:
                return
            # unit complete: quad normalize (all 4 qc at once)
            psos.pop((bi, u))
            if h == 0:
                an = anpool.tile([128, 4, 128], bf16, tag="an", name="an")
                attn[bi, ic] = an
            an = attn[bi, ic]
            rc4 = smpool.tile([128, 4], f32, tag="rc")
            with _dp():
                nc.vector.reciprocal(rc4[:], psoq[:, :, 64:65])
                nc.vector.tensor_tensor(
                    an[:, :, hs:hs + 64], psoq[:, :, 0:64],
                    _bcast_inner(rc4[:], 64), mult,
                )
            if h == 1:
                exb.pop((bi, u - 1), None)
                exb.pop((bi, u), None)
                an = attn.pop((bi, ic))
                OT = ott[bi]
                if TR_MODE == "dma":
                    # XBAR DMA transpose [q, inner] -> [inner, q]; runs on
                    # the (mostly idle) DMA engines instead of PE+PSUM+DVE.
                    for i in range(4):
                        nc.sync.dma_start(
                            OT[:, (ic * 4 + i) * 128:(ic * 4 + i + 1) * 128],
                            an[:, i, :], transpose=True,
                        )
                else:
                    pstq = ps_mm.tile([128, 4, 128], bf16, tag="psm", name="pst")
                    with _pri(TR_PRI):
                        for i in range(4):
                            nc.tensor.transpose(pstq[:, i, :], an[:, i, :], ident[:])
                    with _dp():
                        nc.vector.tensor_copy(
                            OT[:, ic * 512:(ic + 1) * 512], pstq[:],
                        )

        def outproj_chunk(bi, ic, i):
            """Out-projection for one seq-chunk of query-chunk ic."""
            OT = ott[bi]
            sc = 4 * ic + i
            ob = obpool.tile([128, D], bf16, tag="ob")
            if OP_PAIR:
                psp2 = ps_p2.tile([128, 1024], f32, tag="psp2")
                for oc in range(OC):
                    nc.tensor.matmul(
                        psp2[:, oc * 512:(oc + 1) * 512],
                        lhsT=OT[:, ts(sc, 128)],
                        rhs=wo_sb[:, ts(oc, 512)],
                        start=True, stop=True,
                    )
                with _dp():
                    if sc % 2 == 0:
                        nc.scalar.activation(ob[:], psp2[:], Copy)
                    else:
                        nc.vector.tensor_copy(ob[:], psp2[:])
            else:
                # the final ic's op chains run after all scores are done:
                # borrow the idle score-psum pool for 2x deeper overlap
                tail = TAIL_PSS and bi == b - 1 and ic == IC - 1
                for oc in range(OC):
                    pool = ps_sA if tail else ps_mm
                    psp = pool.tile(
                        [128, 512], f32, tag="pss" if tail else "psm", name="psp"
                    )
                    with _pri(OPMM_PRI):
                        nc.tensor.matmul(
                            psp[:],
                            lhsT=OT[:, ts(sc, 128)],
                            rhs=wo_sb[:, ts(oc, 512)],
                            start=True, stop=True,
                        )
                    with _dp():
                        if oc == OP_OC_ACT or ENG_OP1 == "act":
                            nc.scalar.activation(ob[:, ts(oc, 512)], psp[:], Copy)
                        else:
                            nc.vector.tensor_copy(ob[:, ts(oc, 512)], psp[:])
            nc.sync.dma_start(
                out_d[bi * s + sc * 128 : bi * s + (sc + 1) * 128, :], ob[:]
            )

        # ---------------- the global stream ----------------
        # Per global unit gu: weave score-pairs of unit gu with AV chunks
        # of unit gu-2, out-proj chunks of the ic transposed at gu-1, and
        # projection chunks for batch bi+1, so the PE always has non-score
        # work to chew while ACT/DVE drain exps.
        from collections import deque

        load_x(0, fine=True)
        if PROJ_CARRY:
            # preamble = the "next-batch" carry set for batch 0: K, Q ic0,
            # V sc0-11. The rest rides inside batch 0's own window.
            for ic in range(IC):
                proj_qk(0, ic, which=("wk",))
            proj_qk(0, 0, which=("wq",))
            for sc in range(12):
                proj_v(0, sc)
        elif FILL_SPLIT:
            for ic in range(IC):
                proj_qk(0, ic, which=("wk",))
            proj_qk(0, 0, which=("wq",))
        else:
            for ic in range(IC):
                proj_qk(0, ic)
            for sc in range(SC):
                proj_v(0, sc)
        if not PROJ_CARRY and FILL_SPLIT:
            for sc in range(SC):
                proj_v(0, sc)

        op_ready = deque()
        total_units = b * NU
        for gu in range(total_units + 3):
            bi, u = divmod(gu, NU)
            have_s = gu < total_units
            a_gu = gu - 2
            have_a = 0 <= a_gu < total_units
            abi, au = divmod(max(a_gu, 0), NU)
            ops = []
            while op_ready and op_ready[0][0] <= gu:
                ops.append(op_ready.popleft()[1])
            if have_s and u == 0 and bi + 1 < b:
                load_x(bi + 1)

            chunks = []
            if have_s:
                chunks.extend(("s", jp) for jp in range(JP))
            weave_head = []
            if FILL_SPLIT and gu < IC - 1:
                # deferred batch-0 Q chunk: consumer is scores unit 2*(gu+1),
                # two gus away; emit before this gu's scores so its psum
                # drain lands early.
                weave_head.append(("p", (lambda i2=gu + 1: proj_qk(0, i2, which=("wq",)))))
            others = []
            if WEAVE_MODE == 2 and have_s and bi + 1 < b:
                if u < IC:
                    others.append(("p", (lambda b2=bi + 1, i2=u: proj_qk(b2, i2))))
                for sc in range(2 * u, 2 * u + 2):
                    others.append(("p", (lambda b2=bi + 1, s2=sc: proj_v(b2, s2))))
            if WEAVE_MODE == 1:
                others.extend(("o", oi) for oi in ops)
                if have_a:
                    others.extend(("a", qc) for qc in range(4))
            else:
                if have_a:
                    others.extend(("a", qc) for qc in range(4))
                others.extend(("o", oi) for oi in ops)
            weave = []
            si = oi = 0
            if WEAVE_MODE == 4 and others and chunks:
                # proportional: spread others evenly across the chunk stream
                stride = max(1, len(chunks) // len(others))
                for i, c in enumerate(chunks):
                    weave.append(c)
                    if (i + 1) % stride == 0 and oi < len(others):
                        weave.append(others[oi]); oi += 1
                weave.extend(others[oi:]); oi = len(others); si = len(chunks)
            phase = 0 if WEAVE_MODE == 3 else 1
            for i in range(len(chunks) + len(others) - len(weave)):
                take_other = (i % 2 == phase and oi < len(others)) or si >= len(chunks)
                if take_other and oi < len(others):
                    weave.append(others[oi]); oi += 1
                else:
                    weave.append(chunks[si]); si += 1

            for kind, arg in weave_head + weave:
                if kind == "s":
                    scores_exp_chunk(bi, u, arg)
                elif kind == "a":
                    av_norm_chunk(abi, au, arg)
                elif kind == "p":
                    arg()
                else:
                    obi, oic, i = arg
                    outproj_chunk(obi, oic, i)

            if have_a and au % 2 == 1:
                # unit (aic, h1) finished: its 4 seq-chunks are transposed
                aic = au // 2
                lag = OP_LAG
                if OP_LAG_LAST is not None and abi == b - 1 and aic == IC - 1:
                    lag = OP_LAG_LAST
                for i in range(4):
                    ol = lag + (i // 2 if OP_SPREAD else 0)
                    op_ready.append((gu + ol, (abi, aic, i)))
            # projection slices: self-carry (own window) + next-batch carry
            if PROJ_CARRY:
                if have_s:
                    if u == 0:
                        for sc2 in range(12, 16):
                            proj_v(bi, sc2)
                    elif u == 1:
                        proj_qk(bi, 1, which=("wq",))
                    elif u == 3:
                        proj_qk(bi, 2, which=("wq",))
                    elif u == 5:
                        proj_qk(bi, 3, which=("wq",))
                if have_s and bi + 1 < b:
                    if 2 <= u <= 5:
                        proj_qk(bi + 1, u - 2, which=("wk",))
                    if u == 4:
                        for sc2 in range(0, 4):
                            proj_v(bi + 1, sc2)
                    if u == 6:
                        proj_qk(bi + 1, 0, which=("wq",))
                        for sc2 in range(4, 8):
                            proj_v(bi + 1, sc2)
                    if u == 7:
                        for sc2 in range(8, 12):
                            proj_v(bi + 1, sc2)
            elif have_s and bi + 1 < b and WEAVE_MODE != 2:
                if QK_SPREAD:
                    if u < IC:
                        proj_qk(bi + 1, u, which=("wk",))
                    else:
                        proj_qk(bi + 1, u - IC, which=("wq",))
                elif u < IC:
                    proj_qk(bi + 1, u)
                for sc in range(2 * u, 2 * u + 2):
                    proj_v(bi + 1, sc)
        for _, arg in op_ready:
            outproj_chunk(*arg)
    return nc


_NC_CACHE = {}


def _make_nc(b=B, s=S, compile=True):
    from concourse import bacc

    key = (b, s, compile)
    if key in _NC_CACHE:
        return _NC_CACHE[key]
    nc = bacc.Bacc("TRN2", target_bir_lowering=False, debug=False, num_devices=N_CORES)
    build_attention_kernel(nc, b=b, s=s)
    if compile:
        nc.compile()
    _NC_CACHE[key] = nc
    return nc


def _f8(a):
    import ml_dtypes

    return np.asarray(a, np.float32).astype(ml_dtypes.float8_e4m3)


def _wslice_hilo(W, sl):
    """[1024, 128] weight slice -> hi/lo fp8 [128, DC, 128] chunk-major."""
    w = np.asarray(W, np.float32)[:, sl]
    w = np.ascontiguousarray(w.reshape(D // 128, 128, 128).transpose(1, 0, 2)) * WS
    hi = _f8(w)
    lo = _f8(w - hi.astype(np.float32))
    return hi, lo


def kernel(x, Wq, Wk, Wv, Wo, _trace=False):
    import ml_dtypes
    from concourse import bass_utils

    bf16 = ml_dtypes.bfloat16
    x = np.asarray(x, dtype=np.float32)
    b, s, d = x.shape
    flat = np.ascontiguousarray(x.reshape(b * s, d))
    xT = np.ascontiguousarray(flat.T) * XS
    xhi = _f8(xT)
    xlo = _f8(xT - xhi.astype(np.float32))
    ident = np.eye(128, dtype=np.float32).astype(bf16)

    nc = _make_nc(b=b, s=s)

    in_maps = []
    for c in range(N_CORES):
        sl = slice(c * 128, (c + 1) * 128)
        m = {"xhi": xhi, "xlo": xlo, "ident": ident}
        slabs = []
        for wn, W in (("wq", Wq), ("wk", Wk), ("wv", Wv)):
            hi, lo = _wslice_hilo(W, sl)
            slabs += [hi, lo]
        m["w6"] = np.ascontiguousarray(np.stack(slabs, axis=1))
        m["wo"] = np.ascontiguousarray(np.asarray(Wo, np.float32)[sl, :]).astype(bf16)
        in_maps.append(m)

    res = bass_utils.run_bass_kernel_spmd(
        nc, in_maps, core_ids=list(range(N_CORES)), trace=_trace
    )
    acc = np.zeros((b * s, d), np.float32)
    for r in res.results:
        acc += np.asarray(r["out_p"], np.float32)
    out = acc.reshape(b, s, d)
    if _trace:
        kernel._last_results = res
    return out



# revision 45
# speedup vs baseline: 1.0023x; 1.0023x over previous
"""Multi-head attention TRN2 kernel, head-parallel over 8 NeuronCores.

Reference computation (fp32):
    q,k,v = x@Wq, x@Wk, x@Wv          # [B,S,16*64]
    attn  = softmax(q k^T / 8)         # per head
    out   = (attn @ v) @ Wo            # [B,S,1024]

Sharding: tensor-parallel over heads. Core c owns heads (2c, 2c+1):
Wq/Wk/Wv columns [128c:128c+128], Wo rows [128c:128c+128]. Each core
produces a full-shape partial output; the host sums the 8 partials.

v2 design notes (vs the bf16 v1 baseline):
- Projections run as fp8e4m3 DoubleRow matmuls with hi+lo error
  compensation (x = x_hi + x_lo, W = W_hi + W_lo; the lo*lo cross term
  is dropped), giving ~bf16 precision at ~40% of the bf16 PE cost.
- Scores use a q-compensated DoubleRow matmul: lhsT is k8 read through a
  stride-0 pair view, rhs is the (q_hi, q_lo) pair, so s = k8^T(q_hi+q_lo).
  Only the k-side fp8 quantization error survives (~1e-2 of absmax) at
  half the bf16 PE cost.
- exp(s/8) runs on BOTH the ACT engine (native Exp) and the DVE (custom
  8-stage fused op evaluating the minimax polynomial ((a*s+b)^2+c)^16,
  <0.9% rel err over |s/8|<=3.8), splitting the softmax elementwise wall.
  Score PSUM tiles are paired [128,1024] so each exp instruction covers
  two key-chunks (amortizes fixed overheads).
- AV runs "flipped" ([queries, v-dims] output) in bf16: full 128-wide
  partition utilization halves its PE cost vs the v1 orientation, and the
  softmax denominator (ones-column of V) lands per-PARTITION, so the
  normalize is one reciprocal + one tensor_scalar per head-chunk.
- Normalized attention output transposes back to [inner, seq] via PE
  transpose (identity staged from the host) for the bf16 out-proj.
- GPSIMD cannot touch PSUM on TRN2, so all PSUM drains are split across
  ACT (activation-Copy) and DVE; projection work for batch b+1 is
  interleaved into batch b's attention units to keep the PE busy while
  ACT/DVE chew exps.

v3 notes (283us -> 261us on the calibrated timeline):
- The kernel is PSUM-egress-bound: every score/projection/AV element must
  leave PSUM through ACT (1.2G elem/s) or DVE (0.96G), ~430us of combined
  drain+exp work split ~215us/engine. All elementwise ops were widened to
  amortize the fixed access-latency charge: one strided quad-normalize +
  one quad-reciprocal per unit (was 4+4), one 512-wide transpose drain per
  ic (was 4x128 — bf16 PSUM reads get the DVE 2x mode), paired V drains.
- DMA dispatch on SP.SEQ costs ~0.8us per descriptor chain, so the 6 fp8
  weight slabs ship as one dram tensor/DMA (wq+wk first), and x ships as
  one DMA per tensor per batch. Batch 0 loads x in per-ic column windows
  so each K/Q projection chain unblocks as its window lands; first exp
  starts ~10us (was ~20us).
- Score psum tiles (2 bufs x 2 banks) create a score->exp->score cycle of
  ~730ns/pair; the weave + priority tuning (exps high-priority, drains
  natural order, out-proj matmuls slightly late, DVE pairs {1,4,6}) keeps
  both elementwise engines ~93% packed mid-kernel. The final unit's exp
  split is rebalanced (4 DVE/4 ACT) so both engines finish together, and
  the out-proj chunks of an ic spread across 2 gus.
"""

from contextlib import ExitStack
import dataclasses

import numpy as np

HEADS = 16
DH = 64
D = 1024
B = 4
S = 2048
N_CORES = 8
HPC = HEADS // N_CORES  # heads per core = 2

# minimax fit of ((a*y + b)^2 + c)^16 ~ exp(y) over y in [-3.8, 3.8],
# max rel err 0.89%. y = score/8 is folded into C0 (raw scores in).
_PA = 0.044116274207348274
_PB = 0.7133243299023739
_PC = 0.4912647012807798

# fp8 pre-scales: e4m3 subnormals start at 2^-6, so small-sigma data (W has
# sigma=0.02) must be scaled into the normal range before quantization. The
# scales fold into the PSUM drain copies and the exp() scale.
XS = 8.0    # x pre-scale
WS = 32.0   # W pre-scale
QS = 4.0    # q/k drain re-scale
QK_DRAIN = QS / (XS * WS)          # psq -> q/k fp8
V_DRAIN = 1.0 / (XS * WS)          # psv -> V bf16
SSC = 1.0 / (QS * QS)              # score psum = (1/SSC) * raw score
EXP_SCALE = 0.125 * SSC            # ACT exp scale on score psum
C0R = _PA * EXP_SCALE / 2**0.5 * 2**0.5  # == _PA/8*SSC, kept explicit below
C0R = _PA * 0.125 * SSC

# score key-chunk-PAIR indices whose exp runs on the DVE custom op
DVE_JCPS = frozenset({1, 4, 6})  # of 8 pairs per unit
DVE_JCPS_LAST = frozenset({0, 3, 6})  # split for the final batch (no proj drains)
DVE_JCPS_LAST_ALT = None  # if set, odd units of the final batch use this set
OP_LAG_LAST = None  # op lag override for the final ic (None = OP_LAG)
DVE_JCPS_H1 = None  # optional override for h==1 units (they add tr work to DVE)
DVE_JCPS_FIRST = frozenset({0, 3, 5, 6})  # batch-0 early units: DVE has slack
FIRST_UNITS = 2   # how many leading batch-0 units use DVE_JCPS_FIRST
SPLIT_JCPS = frozenset()  # pairs exp'd half-on-ACT half-on-DVE (faster drain)
EXTRA_DVE_EVERY = 4       # every Nth unit sends EXTRA_DVE_PAIRS to DVE too
EXTRA_DVE_PAIRS = frozenset({4})
# engine assignment knobs for the PSUM drains ("act" or "dve")
ENG_V = "dve"
ENG_NORM = "dve"
ENG_TR = "dve"
ENG_OP1 = "dve"
ENG_QLO = "dve"
SCORE_PAIR = True   # pair two key-chunks per PSUM tile / exp instr
PSS_BUFS = 2
PSM_BUFS = 2
HIPRI_EXP = True
SPLIT_PSS = False   # separate score-psum pools for ACT vs DVE exp chains
TR_MODE = "pe"      # "dma" (XBAR) or "pe" (PE transpose + DVE copy)
OP_LAG = 1          # gus between transpose and its out-proj
PSO_QUAD = False    # pack 4 qc AV accumulators into one PSUM bank tile
EXB_BUFS = 5
AN_BUFS = 2
OB_BUFS = 5
PROJ_MODE = "full"   # "full" = 3-term hi/lo comp; "xcomp" = (x_hi+x_lo)*W_hi
VPROJ_MODE = "full"  # same for the V projection
OP_PAIR = False      # pair both oc halves into one [128,1024] psum + 1 drain
WEAVE_MODE = 1       # 0: A then O at odd slots; 1: O first; 2: proj woven too
FILL_SPLIT = False   # defer batch-0 Q ic1..3 into the first gus
DRAIN_PRI = "off"   # "zero": priority 0; "off": natural; int: offset
SCORE_PRI = 100000   # scores hoisted first, relative order preserved
AV_PRI = None        # AV matmul priority ("zero" | int offset | None)
PROJ_PRI = None      # projection matmul priority (negative offset = later)
TR_PRI = None        # PE transpose priority
OPMM_PRI = -100      # out-proj matmul priority
XLOAD_PRI = None     # x-load DMA priority
OP_SPREAD = True    # spread the 4 op chunks of an ic across 2 gus
QK_SPREAD = False    # K projections in gus 0..3, Q in gus 4..7 (1 per gu)
TAIL_PSS = False     # final-ic op psum from the (idle) score pool
PROJ_CARRY = False   # self-carry Q1-3/V12-15 in own window; smaller preamble
OP_OC_ACT = 0        # which out-proj oc half drains on ACT (other goes DVE)
XFINE_ALL = False    # per-ic column-window x DMAs for ALL batches
PSO_BUFS = 2         # AV-accumulator psum bufs (1 frees a bank for psm)
LATE_W = "wo"        # True: wv+wo+ident after batch-0 x; "wo": wo/ident only
DVE_JCPS_FINAL_UNIT = frozenset({0, 2, 4, 6})  # exp split override for the very last unit

_EXP_OP = None


def _exp_op():
    """Register (once) the fused DVE op computing the exp polynomial."""
    global _EXP_OP
    if _EXP_OP is not None:
        return _EXP_OP
    from concourse import dve_ops
    from concourse.dve_spec import C0, C1, C2, Spec, Src0, lower, sq
    from concourse.dve_uop import DveOpSpec

    name = "EXP_POLY16_ANT"
    if name in dve_ops._SUB_OPCODE_FOR_NAME:
        _EXP_OP = next(op for op in dve_ops.OPS if op.name == name)
        return _EXP_OP
    w = Src0 * C0 + C1
    base = sq(w) + C2
    body = sq(sq(sq(sq(base))))
    spec = Spec(
        body=body,
        reference=lambda in0, in1, c0, c1, c2: (
            ((in0.astype(np.float32) * c0 + c1) ** 2 + c2) ** 16
        ),
    )
    opcode = max(dve_ops._SUB_OPCODE_FOR_NAME.values()) + 1
    assert opcode < 0x20
    shas = {
        ver: DveOpSpec(
            name=name, opcode=opcode, uops=lower(spec, ver=ver), rd1_en=False
        ).sha(ver)
        for ver in ("v3", "v4")
    }
    dve_ops._SUB_OPCODE_FOR_NAME[name] = opcode
    op = dve_ops.DveOp(name, spec, subdim=False, uops_sha=shas)
    dve_ops.OPS.append(op)
    dve_ops.CUSTOM_DVE_SPECS[name] = spec
    _EXP_OP = op
    return op


def _pairdim(ap, n=2):
    """Insert a stride-0 dim of size n after the partition dim (broadcast)."""
    return dataclasses.replace(ap, ap=[ap.ap[0], [0, n], *ap.ap[1:]])


def _bcast_inner(ap, n):
    """Append a stride-0 inner dim of size n (broadcast along free)."""
    return dataclasses.replace(ap, ap=[*ap.ap, [0, n]])


def _strided2(ap, stride, count, inner):
    """Reshape a [P, F] AP into [P, count, inner] with the given outer stride."""
    return dataclasses.replace(ap, ap=[ap.ap[0], [stride, count], [1, inner]])


def build_attention_kernel(nc, b=B, s=S):
    """Emit the per-core program. b/s shrinkable for simulator testing."""
    import concourse.bass as bass
    import concourse.tile as tile
    from concourse import mybir

    bf16 = mybir.dt.bfloat16
    f8 = mybir.dt.float8e4
    f32 = mybir.dt.float32
    ts = bass.ts
    DR = mybir.MatmulPerfMode.DoubleRow
    Exp = mybir.ActivationFunctionType.Exp
    Copy = mybir.ActivationFunctionType.Copy
    mult = mybir.AluOpType.mult
    subtract = mybir.AluOpType.subtract
    EXP_OP = _exp_op()

    DC = D // 128          # D chunks of 128 (contraction tiles)
    IC = s // 512          # query chunks of 512 per batch
    JC = s // 128          # key chunks of 128 per batch
    JP = JC // 2           # key-chunk pairs
    SC = s // 128          # seq chunks of 128
    OC = D // 512          # output-dim chunks of 512
    NU = 2 * IC            # attention units per batch: (ic, h)

    xhi_d = nc.dram_tensor("xhi", [D, b * s], f8, kind="ExternalInput").ap()
    xlo_d = nc.dram_tensor("xlo", [D, b * s], f8, kind="ExternalInput").ap()
    # all six fp8 weight slabs in one tensor -> one DMA dispatch
    w6_d = nc.dram_tensor("w6", [128, 6, DC, 128], f8, kind="ExternalInput").ap()
    wo_d = nc.dram_tensor("wo", [128, D], bf16, kind="ExternalInput").ap()
    id_d = nc.dram_tensor("ident", [128, 128], bf16, kind="ExternalInput").ap()
    out_d = nc.dram_tensor("out_p", [b * s, D], bf16, kind="ExternalOutput").ap()

    with tile.TileContext(nc) as tc, ExitStack() as ctx:
        from contextlib import nullcontext

        def _dp():
            if DRAIN_PRI == "off":
                return nullcontext()
            if DRAIN_PRI == "zero":
                return tc.high_priority()
            return tc.high_priority(offset=DRAIN_PRI)

        def _pri(v):
            if v is None:
                return nullcontext()
            if v == "zero":
                return tc.high_priority()
            return tc.high_priority(offset=v)

        wpool = ctx.enter_context(tc.tile_pool(name="weights", bufs=1))
        xpool = ctx.enter_context(tc.tile_pool(name="x", bufs=2))
        qkpool = ctx.enter_context(tc.tile_pool(name="qk", bufs=2))
        vpool = ctx.enter_context(tc.tile_pool(name="v", bufs=2))
        otpool = ctx.enter_context(tc.tile_pool(name="ot", bufs=2))
        expool = ctx.enter_context(tc.tile_pool(name="exp", bufs=EXB_BUFS))
        anpool = ctx.enter_context(tc.tile_pool(name="an", bufs=AN_BUFS))
        smpool = ctx.enter_context(tc.tile_pool(name="small", bufs=8))
        obpool = ctx.enter_context(tc.tile_pool(name="ob", bufs=OB_BUFS))
        # PSUM: 8 banks total. psm (1 bank x2) is shared by projections,
        # out-proj and the attention transpose; pss holds jc-PAIR score
        # tiles (2 banks x2); pso holds the flipped-AV accumulators.
        ps_mm = ctx.enter_context(tc.tile_pool(name="psm", bufs=PSM_BUFS, space="PSUM"))
        if SPLIT_PSS:
            ps_sA = ctx.enter_context(tc.tile_pool(name="pssA", bufs=1, space="PSUM"))
            ps_sD = ctx.enter_context(tc.tile_pool(name="pssD", bufs=1, space="PSUM"))
        else:
            ps_s = ctx.enter_context(tc.tile_pool(name="pss", bufs=PSS_BUFS, space="PSUM"))
            ps_sA = ps_sD = ps_s
        ps_o = ctx.enter_context(tc.tile_pool(name="pso", bufs=PSO_BUFS, space="PSUM"))
        if OP_PAIR:
            ps_p2 = ctx.enter_context(tc.tile_pool(name="psp2", bufs=1, space="PSUM"))

        w6_sb = wpool.tile([128, 6, DC, 128], f8, tag="w6", name="w6")
        # wq+wk first so the batch-0 K/Q chains start before wv lands; the
        # rest of the weights ship after batch-0 x (see preamble) so they
        # stay off the first-matmul critical path.
        nc.sync.dma_start(w6_sb[:, 0:4], w6_d[:, 0:4])
        w_sb = {}
        for i, key in enumerate(
            [(wn, p) for wn in ("wq", "wk", "wv") for p in ("hi", "lo")]
        ):
            w_sb[key] = w6_sb[:, i]
        wo_sb = wpool.tile([128, D], bf16, tag="wo")
        ident = wpool.tile([128, 128], bf16, tag="ident")

        def load_late_weights():
            if LATE_W is True:
                nc.sync.dma_start(w6_sb[:, 4:6], w6_d[:, 4:6])
            nc.sync.dma_start(wo_sb[:], wo_d[:])
            nc.sync.dma_start(ident[:], id_d[:])

        if LATE_W is not True:
            nc.sync.dma_start(w6_sb[:, 4:6], w6_d[:, 4:6])
        if not LATE_W:
            load_late_weights._eager = True
            nc.sync.dma_start(wo_sb[:], wo_d[:])
            nc.sync.dma_start(ident[:], id_d[:])

        # ---------------- per-batch state handles ----------------
        xb = {}      # bi -> (xhi tile, xlo tile)
        qk = {}      # bi -> (QT8 [128,2,s], K8 [128,s])
        vt = {}      # bi -> V
        vps = {}     # bi -> pending V-projection psum pair tile
        ott = {}     # bi -> OT
        exb = {}     # (bi, u) -> exB
        attn = {}    # (bi, ic) -> attn_nat quad tile
        psos = {}    # (bi, u) -> quad AV psum tile

        def _xdram(xd, dc0, ndc, bi):
            """DRAM view [128, ndc, s] of x chunks dc0..dc0+ndc for batch bi."""
            base = xd[dc0 * 128 : dc0 * 128 + 128, bi * s : (bi + 1) * s]
            return dataclasses.replace(
                base, ap=[base.ap[0], [128 * b * s, ndc], base.ap[1]]
            )

        def load_x(bi, fine=False):
            """fine=True: per-ic column-window DMAs so the batch-0 K/Q
            projection chains unblock progressively; else one DMA per tensor
            (single dispatch, latency hidden behind compute)."""
            xh = xpool.tile([128, DC, s], f8, tag="xh")
            xl = xpool.tile([128, DC, s], f8, tag="xl")
            with _pri(XLOAD_PRI):
                if fine:
                    for ic in range(IC):
                        cw = slice(ic * 512, (ic + 1) * 512)
                        for t, d in ((xh, xhi_d), (xl, xlo_d)):
                            src = _xdram(d, 0, DC, bi)
                            src = dataclasses.replace(
                                src,
                                offset=src.offset + ic * 512,
                                ap=[src.ap[0], src.ap[1], [1, 512]],
                            )
                            nc.sync.dma_start(t[:, :, cw], src)
                else:
                    nc.sync.dma_start(xh[:], _xdram(xhi_d, 0, DC, bi))
                    nc.sync.dma_start(xl[:], _xdram(xlo_d, 0, DC, bi))
            xb[bi] = (xh, xl)

        def comp_steps(whi, wlo, xh, xl, cols, full=True):
            """DR step list for the compensated projection contraction.
            full: (W_hi+W_lo)x_hi + W_hi x_lo; else W_hi(x_hi + x_lo) only
            (one fp8 W quantization leg survives, ~7e-3 of absmax)."""
            steps = []
            for dcp in range(DC // 2):
                dsl = slice(2 * dcp, 2 * dcp + 2)
                steps.append((whi[:, dsl, :], xh[:, dsl, cols]))
                if full:
                    steps.append((wlo[:, dsl, :], xh[:, dsl, cols]))
                steps.append((whi[:, dsl, :], xl[:, dsl, cols]))
            return steps

        def proj_qk(bi, ic, which=("wq", "wk")):
            """Q and/or K projection for query-chunk ic of batch bi."""
            xh, xl = xb[bi]
            if bi not in qk:
                QT8 = qkpool.tile([128, 2, s], f8, tag="qt")
                K8 = qkpool.tile([128, s], f8, tag="kt")
                qk[bi] = (QT8, K8)
            QT8, K8 = qk[bi]
            for wn in which:
                psq = ps_mm.tile([128, 512], f32, tag="psm")
                steps = comp_steps(
                    w_sb[wn, "hi"], w_sb[wn, "lo"], xh, xl, ts(ic, 512),
                    full=(PROJ_MODE == "full"),
                )
                n = len(steps)
                with _pri(PROJ_PRI):
                    for i, (lhsT, rhs) in enumerate(steps):
                        nc.tensor.matmul(
                            psq[:], lhsT=lhsT, rhs=rhs,
                            start=(i == 0), stop=(i == n - 1), perf_mode=DR,
                        )
                with _dp():
                    if wn == "wq":
                        nc.scalar.activation(
                            QT8[:, 0, ts(ic, 512)], psq[:], Copy, scale=QK_DRAIN
                        )
                        nc.vector.scalar_tensor_tensor(
                            QT8[:, 1, ts(ic, 512)], psq[:], QK_DRAIN,
                            QT8[:, 0, ts(ic, 512)], mult, subtract,
                        )
                    else:
                        nc.scalar.activation(
                            K8[:, ts(ic, 512)], psq[:], Copy, scale=QK_DRAIN
                        )

        def proj_v(bi, sc):
            """V projection for key-chunk sc of batch bi (natural layout).
            sc pairs share one psum tile; the drain fires on the odd sc as a
            single 256-row strided copy."""
            xh, xl = xb[bi]
            if sc == 0:
                V = vpool.tile([128, SC, 130], bf16, tag="v")
                nc.vector.memset(V[:, :, 64:65], 1.0)
                nc.vector.memset(V[:, :, 129:130], 1.0)
                vt[bi] = V
            V = vt[bi]
            if sc % 2 == 0:
                vps[bi] = ps_mm.tile([128, 256], f32, tag="psm", name="psv")
            psv = vps[bi]
            half = sc % 2
            steps = []
            for dcp in range(DC // 2):
                dsl = slice(2 * dcp, 2 * dcp + 2)
                steps.append((xh[:, dsl, ts(sc, 128)], w_sb["wv", "hi"][:, dsl, :]))
                if VPROJ_MODE == "full":
                    steps.append((xh[:, dsl, ts(sc, 128)], w_sb["wv", "lo"][:, dsl, :]))
                steps.append((xl[:, dsl, ts(sc, 128)], w_sb["wv", "hi"][:, dsl, :]))
            n = len(steps)
            with _pri(PROJ_PRI):
                for i, (lhsT, rhs) in enumerate(steps):
                    nc.tensor.matmul(
                        psv[:, half * 128 : half * 128 + 128], lhsT=lhsT, rhs=rhs,
                        start=(i == 0), stop=(i == n - 1), perf_mode=DR,
                    )
            if half == 0:
                return
            # one strided scaled copy for both sc chunks:
            # psum [128, (2 sc, 2, 64)] -> V[:, sc-1:sc+1, (0:64, 65:129)]
            vps.pop(bi)
            vv = V[:, sc - 1 : sc + 1, 0:129]
            vv = dataclasses.replace(
                vv, ap=[vv.ap[0], vv.ap[1], [65, 2], [1, 64]]
            )
            pv = dataclasses.replace(
                psv[:], ap=[psv[:].ap[0], [128, 2], [64, 2], [1, 64]]
            )
            with _dp():
                if ENG_V == "dve":
                    nc.vector.tensor_scalar(vv, pv, V_DRAIN, None, mult)
                else:
                    nc.scalar.activation(vv, pv, Copy, scale=V_DRAIN)

        def scores_exp_chunk(bi, u, jp):
            """One score-pair (2 key-chunks) + its exp, for unit u=(ic,h)."""
            ic, h = divmod(u, 2)
            QT8, K8 = qk[bi]
            hs = h * 64
            if jp == 0:
                exB = expool.tile([128, JP, 1024], bf16, tag="ex", name=f"ex{bi}_{u}")
                exb[bi, u] = exB
            exB = exb[bi, u]
            from contextlib import nullcontext

            def pri():
                if HIPRI_EXP is True:
                    return tc.high_priority()
                if HIPRI_EXP:
                    return tc.high_priority(offset=HIPRI_EXP)
                return nullcontext()
            if SCORE_PAIR:
                dve = jp in DVE_JCPS
                pool = ps_sD if dve else ps_sA
                pss = pool.tile([128, 1024], f32, tag="pssD" if (dve and SPLIT_PSS) else "pss")
                if SCORE_PRI is None:
                    spri = nullcontext()
                elif SCORE_PRI == "zero":
                    spri = tc.high_priority()
                else:
                    spri = tc.high_priority(offset=SCORE_PRI)
                with spri:
                    for half in range(2):
                        jc = 2 * jp + half
                        nc.tensor.matmul(
                            pss[:, half * 512:(half + 1) * 512],
                            lhsT=_pairdim(K8[hs:hs + 64, ts(jc, 128)]),
                            rhs=QT8[hs:hs + 64, :, ts(ic, 512)],
                            start=True, stop=True, perf_mode=DR,
                        )
                jset = DVE_JCPS_LAST if bi == b - 1 else DVE_JCPS
                if bi == b - 1 and u % 2 == 1 and DVE_JCPS_LAST_ALT is not None:
                    jset = DVE_JCPS_LAST_ALT
                if (bi == b - 1 and u == NU - 1
                        and DVE_JCPS_FINAL_UNIT is not None):
                    jset = DVE_JCPS_FINAL_UNIT
                if h == 1 and DVE_JCPS_H1 is not None:
                    jset = DVE_JCPS_H1
                if bi == 0 and u < FIRST_UNITS and DVE_JCPS_FIRST is not None:
                    jset = DVE_JCPS_FIRST
                if EXTRA_DVE_EVERY and (bi * NU + u) % EXTRA_DVE_EVERY == 0:
                    jset = frozenset(jset) | EXTRA_DVE_PAIRS
                with pri():
                    if jp in SPLIT_JCPS:
                        nc.scalar.activation(
                            exB[:, jp, 0:512], pss[:, 0:512], Exp, scale=EXP_SCALE
                        )
                        nc.vector._custom_dve(
                            EXP_OP, out=exB[:, jp, 512:1024], in0=pss[:, 512:1024],
                            s0=C0R, s1=_PB, imm2=_PC,
                        )
                    elif jp in jset:
                        nc.vector._custom_dve(
                            EXP_OP, out=exB[:, jp, :], in0=pss[:],
                            s0=C0R, s1=_PB, imm2=_PC,
                        )
                    else:
                        nc.scalar.activation(
                            exB[:, jp, :], pss[:], Exp, scale=EXP_SCALE
                        )
            else:
                for half in range(2):
                    jc = 2 * jp + half
                    pss = ps_s.tile([128, 512], f32, tag="pss")
                    nc.tensor.matmul(
                        pss[:],
                        lhsT=_pairdim(K8[hs:hs + 64, ts(jc, 128)]),
                        rhs=QT8[hs:hs + 64, :, ts(ic, 512)],
                        start=True, stop=True, perf_mode=DR,
                    )
                    jset = DVE_JCPS_LAST if bi == b - 1 else DVE_JCPS
                    with pri():
                        if jp in jset:
                            nc.vector._custom_dve(
                                EXP_OP,
                                out=exB[:, jp, half * 512:(half + 1) * 512],
                                in0=pss[:], s0=C0R, s1=_PB, imm2=_PC,
                            )
                        else:
                            nc.scalar.activation(
                                exB[:, jp, half * 512:(half + 1) * 512],
                                pss[:], Exp, scale=EXP_SCALE,
                            )

        def av_norm_chunk(bi, u, qc):
            """Flipped AV for one query sub-chunk of unit u. At qc==3 the
            whole unit normalizes in one strided quad op; at (h==1, qc==3)
            the finished [q, inner] ic-block transposes into OT via 4 PE
            transposes + one 512-wide drain."""
            ic, h = divmod(u, 2)
            V = vt[bi]
            exB = exb[bi, u]
            hs = h * 64
            if ic == 0 and h == 0 and qc == 0:
                OT = otpool.tile([128, s], bf16, tag="ot")
                ott[bi] = OT
            if qc == 0:
                psoq = ps_o.tile([128, 4, 65], f32, tag="pso", name="psoq")
                psos[bi, u] = psoq
            psoq = psos[bi, u]
            with _pri(AV_PRI):
                for jc in range(JC):
                    jp, half = divmod(jc, 2)
                    nc.tensor.matmul(
                        psoq[:, qc, :],
                        lhsT=exB[:, jp, half * 512 + qc * 128: half * 512 + (qc + 1) * 128],
                        rhs=V[:, jc, 65 * h : 65 * h + 65],
                        start=(jc == 0), stop=(jc == JC - 1),
                    )
            if qc < 3:
                return
            # unit complete: quad normalize (all 4 qc at once)
            psos.pop((bi, u))
            if h == 0:
                an = anpool.tile([128, 4, 128], bf16, tag="an", name="an")
                attn[bi, ic] = an
            an = attn[bi, ic]
            rc4 = smpool.tile([128, 4], f32, tag="rc")
            with _dp():
                nc.vector.reciprocal(rc4[:], psoq[:, :, 64:65])
                nc.vector.tensor_tensor(
                    an[:, :, hs:hs + 64], psoq[:, :, 0:64],
                    _bcast_inner(rc4[:], 64), mult,
                )
            if h == 1:
                exb.pop((bi, u - 1), None)
                exb.pop((bi, u), None)
                an = attn.pop((bi, ic))
                OT = ott[bi]
                if TR_MODE == "dma":
                    # XBAR DMA transpose [q, inner] -> [inner, q]; runs on
                    # the (mostly idle) DMA engines instead of PE+PSUM+DVE.
                    for i in range(4):
                        nc.sync.dma_start(
                            OT[:, (ic * 4 + i) * 128:(ic * 4 + i + 1) * 128],
                            an[:, i, :], transpose=True,
                        )
                else:
                    pstq = ps_mm.tile([128, 4, 128], bf16, tag="psm", name="pst")
                    with _pri(TR_PRI):
                        for i in range(4):
                            nc.tensor.transpose(pstq[:, i, :], an[:, i, :], ident[:])
                    with _dp():
                        nc.vector.tensor_copy(
                            OT[:, ic * 512:(ic + 1) * 512], pstq[:],
                        )

        def outproj_chunk(bi, ic, i):
            """Out-projection for one seq-chunk of query-chunk ic."""
            OT = ott[bi]
            sc = 4 * ic + i
            ob = obpool.tile([128, D], bf16, tag="ob")
            if OP_PAIR:
                psp2 = ps_p2.tile([128, 1024], f32, tag="psp2")
                for oc in range(OC):
                    nc.tensor.matmul(
                        psp2[:, oc * 512:(oc + 1) * 512],
                        lhsT=OT[:, ts(sc, 128)],
                        rhs=wo_sb[:, ts(oc, 512)],
                        start=True, stop=True,
                    )
                with _dp():
                    if sc % 2 == 0:
                        nc.scalar.activation(ob[:], psp2[:], Copy)
                    else:
                        nc.vector.tensor_copy(ob[:], psp2[:])
            else:
                # the final ic's op chains run after all scores are done:
                # borrow the idle score-psum pool for 2x deeper overlap
                tail = TAIL_PSS and bi == b - 1 and ic == IC - 1
                for oc in range(OC):
                    pool = ps_sA if tail else ps_mm
                    psp = pool.tile(
                        [128, 512], f32, tag="pss" if tail else "psm", name="psp"
                    )
                    with _pri(OPMM_PRI):
                        nc.tensor.matmul(
                            psp[:],
                            lhsT=OT[:, ts(sc, 128)],
                            rhs=wo_sb[:, ts(oc, 512)],
                            start=True, stop=True,
                        )
                    with _dp():
                        if oc == OP_OC_ACT or ENG_OP1 == "act":
                            nc.scalar.activation(ob[:, ts(oc, 512)], psp[:], Copy)
                        else:
                            nc.vector.tensor_copy(ob[:, ts(oc, 512)], psp[:])
            nc.sync.dma_start(
                out_d[bi * s + sc * 128 : bi * s + (sc + 1) * 128, :], ob[:]
            )

        # ---------------- the global stream ----------------
        # Per global unit gu: weave score-pairs of unit gu with AV chunks
        # of unit gu-2, out-proj chunks of the ic transposed at gu-1, and
        # projection chunks for batch bi+1, so the PE always has non-score
        # work to chew while ACT/DVE drain exps.
        from collections import deque

        load_x(0, fine=True)
        if LATE_W:
            load_late_weights()
        if PROJ_CARRY:
            # preamble = the "next-batch" carry set for batch 0: K, Q ic0,
            # V sc0-11. The rest rides inside batch 0's own window.
            for ic in range(IC):
                proj_qk(0, ic, which=("wk",))
            proj_qk(0, 0, which=("wq",))
            for sc in range(12):
                proj_v(0, sc)
        elif FILL_SPLIT:
            for ic in range(IC):
                proj_qk(0, ic, which=("wk",))
            proj_qk(0, 0, which=("wq",))
        else:
            for ic in range(IC):
                proj_qk(0, ic)
            for sc in range(SC):
                proj_v(0, sc)
        if not PROJ_CARRY and FILL_SPLIT:
            for sc in range(SC):
                proj_v(0, sc)

        op_ready = deque()
        total_units = b * NU
        for gu in range(total_units + 3):
            bi, u = divmod(gu, NU)
            have_s = gu < total_units
            a_gu = gu - 2
            have_a = 0 <= a_gu < total_units
            abi, au = divmod(max(a_gu, 0), NU)
            ops = []
            while op_ready and op_ready[0][0] <= gu:
                ops.append(op_ready.popleft()[1])
            if have_s and u == 0 and bi + 1 < b:
                load_x(bi + 1, fine=XFINE_ALL)

            chunks = []
            if have_s:
                chunks.extend(("s", jp) for jp in range(JP))
            weave_head = []
            if FILL_SPLIT and gu < IC - 1:
                # deferred batch-0 Q chunk: consumer is scores unit 2*(gu+1),
                # two gus away; emit before this gu's scores so its psum
                # drain lands early.
                weave_head.append(("p", (lambda i2=gu + 1: proj_qk(0, i2, which=("wq",)))))
            others = []
            if WEAVE_MODE == 2 and have_s and bi + 1 < b:
                if u < IC:
                    others.append(("p", (lambda b2=bi + 1, i2=u: proj_qk(b2, i2))))
                for sc in range(2 * u, 2 * u + 2):
                    others.append(("p", (lambda b2=bi + 1, s2=sc: proj_v(b2, s2))))
            if WEAVE_MODE == 1:
                others.extend(("o", oi) for oi in ops)
                if have_a:
                    others.extend(("a", qc) for qc in range(4))
            else:
                if have_a:
                    others.extend(("a", qc) for qc in range(4))
                others.extend(("o", oi) for oi in ops)
            weave = []
            si = oi = 0
            if WEAVE_MODE == 4 and others and chunks:
                # proportional: spread others evenly across the chunk stream
                stride = max(1, len(chunks) // len(others))
                for i, c in enumerate(chunks):
                    weave.append(c)
                    if (i + 1) % stride == 0 and oi < len(others):
                        weave.append(others[oi]); oi += 1
                weave.extend(others[oi:]); oi = len(others); si = len(chunks)
            phase = 0 if WEAVE_MODE == 3 else 1
            for i in range(len(chunks) + len(others) - len(weave)):
                take_other = (i % 2 == phase and oi < len(others)) or si >= len(chunks)
                if take_other and oi < len(others):
                    weave.append(others[oi]); oi += 1
                else:
                    weave.append(chunks[si]); si += 1

            for kind, arg in weave_head + weave:
                if kind == "s":
                    scores_exp_chunk(bi, u, arg)
                elif kind == "a":
                    av_norm_chunk(abi, au, arg)
                elif kind == "p":
                    arg()
                else:
                    obi, oic, i = arg
                    outproj_chunk(obi, oic, i)

            if have_a and au % 2 == 1:
                # unit (aic, h1) finished: its 4 seq-chunks are transposed
                aic = au // 2
                lag = OP_LAG
                if OP_LAG_LAST is not None and abi == b - 1 and aic == IC - 1:
                    lag = OP_LAG_LAST
                for i in range(4):
                    ol = lag + (i // 2 if OP_SPREAD else 0)
                    op_ready.append((gu + ol, (abi, aic, i)))
            # projection slices: self-carry (own window) + next-batch carry
            if PROJ_CARRY:
                if have_s:
                    if u == 0:
                        for sc2 in range(12, 16):
                            proj_v(bi, sc2)
                    elif u == 1:
                        proj_qk(bi, 1, which=("wq",))
                    elif u == 3:
                        proj_qk(bi, 2, which=("wq",))
                    elif u == 5:
                        proj_qk(bi, 3, which=("wq",))
                if have_s and bi + 1 < b:
                    if 2 <= u <= 5:
                        proj_qk(bi + 1, u - 2, which=("wk",))
                    if u == 4:
                        for sc2 in range(0, 4):
                            proj_v(bi + 1, sc2)
                    if u == 6:
                        proj_qk(bi + 1, 0, which=("wq",))
                        for sc2 in range(4, 8):
                            proj_v(bi + 1, sc2)
                    if u == 7:
                        for sc2 in range(8, 12):
                            proj_v(bi + 1, sc2)
            elif have_s and bi + 1 < b and WEAVE_MODE != 2:
                if QK_SPREAD:
                    if u < IC:
                        proj_qk(bi + 1, u, which=("wk",))
                    else:
                        proj_qk(bi + 1, u - IC, which=("wq",))
                elif u < IC:
                    proj_qk(bi + 1, u)
                for sc in range(2 * u, 2 * u + 2):
                    proj_v(bi + 1, sc)
        for _, arg in op_ready:
            outproj_chunk(*arg)
    return nc


_NC_CACHE = {}


def _make_nc(b=B, s=S, compile=True):
    from concourse import bacc

    key = (b, s, compile)
    if key in _NC_CACHE:
        return _NC_CACHE[key]
    nc = bacc.Bacc("TRN2", target_bir_lowering=False, debug=False, num_devices=N_CORES)
    build_attention_kernel(nc, b=b, s=s)
    if compile:
        nc.compile()
    _NC_CACHE[key] = nc
    return nc


def _f8(a):
    import ml_dtypes

    return np.asarray(a, np.float32).astype(ml_dtypes.float8_e4m3)


def _wslice_hilo(W, sl):
    """[1024, 128] weight slice -> hi/lo fp8 [128, DC, 128] chunk-major."""
    w = np.asarray(W, np.float32)[:, sl]
    w = np.ascontiguousarray(w.reshape(D // 128, 128, 128).transpose(1, 0, 2)) * WS
    hi = _f8(w)
    lo = _f8(w - hi.astype(np.float32))
    return hi, lo


def kernel(x, Wq, Wk, Wv, Wo, _trace=False):
    import ml_dtypes
    from concourse import bass_utils

    bf16 = ml_dtypes.bfloat16
    x = np.asarray(x, dtype=np.float32)
    b, s, d = x.shape
    flat = np.ascontiguousarray(x.reshape(b * s, d))
    xT = np.ascontiguousarray(flat.T) * XS
    xhi = _f8(xT)
    xlo = _f8(xT - xhi.astype(np.float32))
    ident = np.eye(128, dtype=np.float32).astype(bf16)

    nc = _make_nc(b=b, s=s)

    in_maps = []
    for c in range(N_CORES):
        sl = slice(c * 128, (c + 1) * 128)
        m = {"xhi": xhi, "xlo": xlo, "ident": ident}
        slabs = []
        for wn, W in (("wq", Wq), ("wk", Wk), ("wv", Wv)):
            hi, lo = _wslice_hilo(W, sl)
            slabs += [hi, lo]
        m["w6"] = np.ascontiguousarray(np.stack(slabs, axis=1))
        m["wo"] = np.ascontiguousarray(np.asarray(Wo, np.float32)[sl, :]).astype(bf16)
        in_maps.append(m)

    res = bass_utils.run_bass_kernel_spmd(
        nc, in_maps, core_ids=list(range(N_CORES)), trace=_trace
    )
    acc = np.zeros((b * s, d), np.float32)
    for r in res.results:
        acc += np.asarray(r["out_p"], np.float32)
    out = acc.reshape(b, s, d)
    if _trace:
        kernel._last_results = res
    return out

